# revision 22
# baseline (speedup 1.0000x reference)
"""MoE post-processing MLP kernel for Trainium2 (8 NeuronCores).

Strategy: expert-parallel sharding. Each core is assigned one chunk of
samples routed to a single expert (K=8 experts ~= 8 cores for uniform
routing). The host gathers/permutes samples by expert and the device
runs a dense 3-layer MLP entirely in fp16 (fp32 PSUM accumulation).

Posenc on device: u36 = R^T @ fpv computes all angle/2pi rows (+phase
+0.5) on the PE; v = floormod(u,1)-0.5 on Vector (single dual-op
tensor_scalar); s36 = Sin(2pi*v) on Scalar (LUT domain [-pi,pi]).
To keep fp16 phase error small at the largest scales, the host uploads
x4~ = mod(4*view/2pi,1)-0.5 (3 extra rows/block); the m=4 row uses it
with coefficient 1 and m=8 with coefficient 2 (2*x mod 1 == (2x) mod 1).

Device layout (C=8704 samples: 8 pair-packed 512-col tiles + one
256-col tail tile; the tail group runs FIRST for fast pipeline fill):
  h0 = relu(W0a^T@fpv + W0s^T@s36 + b0); h1 = relu(W1^T@h0 + b1)
  y = W2^T@h1 + b2
relu0 on Scalar, relu1 split Scalar/Vector, y-copy on Vector.
DMA rings: SP carries the small fill set (tail tile + weights) and the
outputs; the two bulk fpv halves go one each on the GpSimd software
ring and the Scalar hardware ring, so no consumer ever waits behind an
unrelated bulk transfer (ring waits are cumulative per queue).
A zero-weight matmul chain into the first PSUM group keeps the PE busy
from program start so its clock ramps to max before the real work.
"""

import numpy as np

K = 8
WID = 64
D = 32
NT = 512            # full-tile matmul moving dim (one fp32 PSUM bank)
NFULL = 8           # full tiles (1024 samples each, pair-packed)
TNT = 256           # tail-tile moving dim (512 samples)
C = NFULL * 2 * NT + 2 * TNT     # 8704 samples per core-chunk
COLS = NFULL * NT + TNT          # 4352 device columns
TWO_PI = float(2.0 * np.pi)
RB = 42             # input rows per block: feat32 pos3 view3 ones1 x4~3
CMAGIC = 12582912.0  # 1.5 * 2**23, round-to-nearest magic constant

# processing order: tail group first (tiny -> fast fill), then the four
# 1024-wide pair groups; (col0, fw) in device column space
PGROUPS = [(0, 1024), (1024, 1024), (2048, 1024), (3072, 1024),
           (4096, 256)]

# W0 row indices (DIN=74 layout: feat 0:32, posenc(pos,2) 32:47,
# posenc(view,4) 47:74) for the identity part and the sin part.
_W0A_ROWS = list(range(32)) + [32, 33, 34] + [47, 48, 49]
_W0S_ROWS = (list(range(35, 41)) + list(range(50, 62))
             + list(range(41, 47)) + list(range(62, 74)))

_PREP = None  # compiled Bass program, built once per process
_LAST_IN_MAPS = None  # stashed for external profiling harnesses


def _build_R():
    """R' [RB, 36]: u = m*x/2pi + 0.25*phase + 0.5 per angle column.
    Rows: feat(unused) 0:32, pos 32:35, view 35:38, ones 38, x4~ 39:42.
    The m=4/m=8 view columns read the host-prereduced x4~ rows."""
    Rp = np.zeros((RB, 36), np.float32)
    col = 0
    for p in range(2):
        base_phase = 0.25 * p
        for m in (1.0, 2.0):
            for c in range(3):
                Rp[32 + c, col] = m / TWO_PI
                Rp[38, col] = base_phase
                col += 1
        for m in (1.0, 2.0):
            for c in range(3):
                Rp[35 + c, col] = m / TWO_PI
                Rp[38, col] = base_phase
                col += 1
        for coef in (1.0, 2.0):
            for c in range(3):
                Rp[39 + c, col] = coef
                # x4~ stored -0.5: u = coef*(x4+0.5)+... fold into phase
                Rp[38, col] = base_phase + 0.5 * coef
                col += 1
    return Rp


def _build_program():
    import concourse.bacc as bacc
    import concourse.mybir as mybir
    from concourse.tile import TileContext

    F32, F16 = mybir.dt.float32, mybir.dt.float16
    AF = mybir.ActivationFunctionType
    ALU = mybir.AluOpType

    nc = bacc.Bacc("TRN2", target_bir_lowering=False, debug=False,
                   num_devices=8)

    fpv_d = nc.dram_tensor("fpv", [2 * RB, COLS], F16,
                           kind="ExternalInput").ap()
    wall_d = nc.dram_tensor("wall", [128, 592], F16,
                            kind="ExternalInput").ap()
    bias_d = nc.dram_tensor("bias", [128, 3], F32, kind="ExternalInput").ap()
    y_d = nc.dram_tensor("y", [64, COLS], F16, kind="ExternalOutput").ap()

    with TileContext(nc) as tc:
        with (tc.tile_pool(name="w", bufs=1) as wp,
              tc.tile_pool(name="fp", bufs=1) as fpool,
              tc.tile_pool(name="io", bufs=3) as io,
              tc.tile_pool(name="psu", bufs=2, space="PSUM") as psu,
              tc.tile_pool(name="ps0", bufs=1, space="PSUM") as ps0,
              tc.tile_pool(name="ps1", bufs=1, space="PSUM") as ps1,
              tc.tile_pool(name="psy", bufs=1, space="PSUM") as psy):
            wall = wp.tile([128, 592], F16)
            biasw = wp.tile([128, 3], F32)
            dummy = wp.tile([128, 512], F16)
            ft_A = fpool.tile([2 * RB, 2048], F16)
            ft_B = fpool.tile([2 * RB, 2048], F16)
            ft_T = fpool.tile([2 * RB, 256], F16)

            def fpv_src(c0, fw):
                if c0 >= 4096:
                    return ft_T[:, 0:fw]
                t = ft_A if c0 < 2048 else ft_B
                return t[:, c0 % 2048:c0 % 2048 + fw]

            # DMA rings: SP = fill set + outputs; GpSimd software ring =
            # first bulk half; Scalar hardware ring = second bulk half.
            nc.gpsimd.memset(dummy[:], 0.0)
            nc.gpsimd.dma_start(out=ft_A[:], in_=fpv_d[:, 0:2048])
            nc.gpsimd.dma_start(out=ft_B[:], in_=fpv_d[:, 2048:4096])
            nc.sync.dma_start(out=ft_T[:], in_=fpv_d[:, 4096:4352])
            nc.sync.dma_start(out=wall[:], in_=wall_d[:, :])
            nc.sync.dma_start(out=biasw[:], in_=bias_d[:, :])

            Rt = wall[0:2 * RB, 0:72]
            W0at = wall[0:2 * RB, 72:200]
            W0st = wall[0:72, 200:328]
            W1t = wall[0:128, 328:456]
            W2t = wall[0:128, 456:520]
            negI = wall[0:72, 520:592]
            b0t = biasw[0:128, 0:1]
            b1t = biasw[0:128, 1:2]
            b2t = biasw[0:64, 2:3]

            # Software-pipelined emission with one-group lookahead: the
            # next group's filler/R/W0a matmuls (data-independent) are
            # emitted into the PE stream before this group's W2, and its
            # round ops before this group's y-copy, so the PE and Vector
            # queues keep working through the relu1->W2 dependency window.
            state = {}

            def emit_uphase(gi):
                gc0, fw = PGROUPS[gi]
                s36 = io.tile([72, fw], F16, name="s36")
                ups = []
                for off in range(0, fw, NT):
                    w = min(NT, fw - off)
                    up = psu.tile([72, NT], F32, name="up")
                    vt = io.tile([72, NT], F16, name="vt")
                    nd = 14 if (gi == 0 and off == 0) else 1
                    for d in range(nd):
                        nc.tensor.matmul(out=up[:, 0:NT - d],
                                         lhsT=dummy[:, 0:72],
                                         rhs=dummy[:, 0:NT - d],
                                         start=(d == 0), stop=False)
                    nc.tensor.matmul(out=up[:, 0:w], lhsT=Rt,
                                     rhs=fpv_src(gc0 + off, w),
                                     start=False, stop=False)
                    nc.vector.tensor_scalar(out=vt[:, 0:w], in0=up[:, 0:w],
                                            scalar1=CMAGIC, scalar2=CMAGIC,
                                            op0=ALU.add, op1=ALU.subtract)
                    ups.append((up, vt, off, w))
                state[gi] = (s36, ups)

            def emit_w0a(gi, h0p):
                gc0, fw = PGROUPS[gi]
                for off in range(0, fw, NT):
                    w = min(NT, fw - off)
                    js = slice(off, off + w)
                    nc.tensor.matmul(out=h0p[:, js], lhsT=W0at,
                                     rhs=fpv_src(gc0 + off, w),
                                     start=True, stop=False)

            for gi, (gc0, fw) in enumerate(PGROUPS):
                if gi == 0:
                    emit_uphase(0)
                    h0p = ps0.tile([128, fw], F32, name="h0p")
                    emit_w0a(0, h0p)
                    state[0] = state[0] + (h0p,)
                s36, ups, h0p = state.pop(gi)
                h0t = io.tile([128, fw], F16, name="h0t")
                h1t = io.tile([128, fw], F16, name="h1t")
                yt = io.tile([64, fw], F16, name="yt")
                h1p = ps1.tile([128, fw], F32, name="h1p")
                yp = psy.tile([64, fw], F32, name="yp")

                for up, vt, off, w in ups:
                    js = slice(off, off + w)
                    nc.tensor.matmul(out=up[:, 0:w], lhsT=negI,
                                     rhs=vt[:, 0:w], start=False, stop=True)
                    nc.scalar.activation(s36[:, js], up[:, 0:w], AF.Sin,
                                         bias=0.0, scale=TWO_PI)
                for off in range(0, fw, NT):
                    w = min(NT, fw - off)
                    js = slice(off, off + w)
                    nc.tensor.matmul(out=h0p[:, js], lhsT=W0st,
                                     rhs=s36[:, js], start=False, stop=True)
                nc.scalar.activation(h0t[:], h0p[:], AF.Relu,
                                     bias=b0t, scale=1.0)
                for off in range(0, fw, NT):
                    w = min(NT, fw - off)
                    js = slice(off, off + w)
                    nc.tensor.matmul(out=h1p[:, js], lhsT=W1t,
                                     rhs=h0t[:, js], start=True, stop=True)
                nc.vector.tensor_scalar(out=h1t[:], in0=h1p[:],
                                        scalar1=b1t, scalar2=0.0,
                                        op0=ALU.add, op1=ALU.max)
                if gi + 1 < len(PGROUPS):
                    emit_uphase(gi + 1)
                    nh0p = ps0.tile([128, PGROUPS[gi + 1][1]], F32,
                                    name="h0p")
                    emit_w0a(gi + 1, nh0p)
                    state[gi + 1] = state[gi + 1] + (nh0p,)
                for off in range(0, fw, NT):
                    w = min(NT, fw - off)
                    js = slice(off, off + w)
                    nc.tensor.matmul(out=yp[:, js], lhsT=W2t,
                                     rhs=h1t[:, js], start=True, stop=True)
                ys = (fw * 62 // 100) & ~31
                nc.scalar.activation(yt[:, 0:ys], yp[:, 0:ys],
                                     AF.Identity, bias=b2t, scale=1.0)
                nc.vector.tensor_scalar(out=yt[:, ys:fw], in0=yp[:, ys:fw],
                                        scalar1=b2t, scalar2=None,
                                        op0=ALU.add)
                nc.sync.dma_start(out=y_d[:, gc0:gc0 + fw], in_=yt[:])

    nc.compile()
    return nc


def _get_program():
    global _PREP
    if _PREP is None:
        _PREP = _build_program()
    return _PREP


def _pack_weights(W0, b0, W1, b1, W2, b2):
    """Per-expert [128, 592] fp16 weight wall + [128, 3] f32 biases."""
    W0a = np.zeros((RB, 64), np.float32)
    W0a[0:38] = W0[_W0A_ROWS]
    W0s = W0[_W0S_ROWS].astype(np.float32)          # [36, 64]
    Rp = _build_R()
    wall = np.zeros((128, 592), np.float16)
    wall[0:RB, 0:36] = Rp
    wall[RB:2 * RB, 36:72] = Rp
    wall[0:RB, 72:136] = W0a
    wall[RB:2 * RB, 136:200] = W0a
    wall[0:36, 200:264] = W0s
    wall[36:72, 264:328] = W0s
    wall[0:64, 328:392] = W1
    wall[64:128, 392:456] = W1
    wall[0:64, 456:488] = W2
    wall[64:128, 488:520] = W2
    wall[0:72, 520:592] = -np.eye(72, dtype=np.float16)
    bias = np.zeros((128, 3), np.float32)
    bias[:, 0] = np.concatenate([b0, b0])
    bias[:, 1] = np.concatenate([b1, b1])
    bias[0:64, 2] = np.concatenate([b2, b2])
    return wall, bias


def _pack_cols(data, n):
    """[R, C-samples] -> [2R, COLS] pair-packed device layout."""
    R = data.shape[0]
    full = data[:, :NFULL * 2 * NT].reshape(R, NFULL, 2, NT)
    fullp = np.concatenate([full[:, :, 0], full[:, :, 1]],
                           axis=0).reshape(2 * R, NFULL * NT)
    tail = data[:, NFULL * 2 * NT:].reshape(R, 1, 2, TNT)
    tailp = np.concatenate([tail[:, :, 0], tail[:, :, 1]],
                           axis=0).reshape(2 * R, TNT)
    return np.concatenate([fullp, tailp], axis=1)


def _unpack_cols(y):
    """[64, COLS] device layout -> [32, C] sample order."""
    yf = y[:, :NFULL * NT].reshape(64, NFULL, NT)
    full = np.stack([yf[0:32], yf[32:64]], axis=2).reshape(32, NFULL * 2 * NT)
    ytl = y[:, NFULL * NT:].reshape(64, 1, TNT)
    tail = np.stack([ytl[0:32], ytl[32:64]], axis=2).reshape(32, 2 * TNT)
    return np.concatenate([full, tail], axis=1)


def kernel(idxs, positions, viewdirs, features, W0, b0, W1, b1, W2, b2):
    from concourse.bass_utils import run_bass_kernel_spmd

    N = idxs.shape[0]
    idx = idxs.reshape(-1).astype(np.int64)
    out = np.zeros((N, D), np.float32)

    # Route: list of (expert, sample-index-array) chunks of <= C samples.
    chunks = []
    for k in range(K):
        sel = np.nonzero(idx == k)[0]
        for lo in range(0, len(sel), C):
            chunks.append((k, sel[lo:lo + C]))

    walls = [_pack_weights(W0[k], b0[k], W1[k], b1[k], W2[k], b2[k])
             for k in range(K)]

    nc = _get_program()
    zero_in = None
    for inv in range(0, len(chunks), 8):
        batch = chunks[inv:inv + 8]
        in_maps = []
        for ci in range(8):
            if ci < len(batch):
                k, sel = batch[ci]
                n = len(sel)
                fpv = np.zeros((RB, C), np.float16)
                fpv[0:32, :n] = features[sel].T
                fpv[32:35, :n] = positions[sel].T
                fpv[35:38, :n] = viewdirs[sel].T
                fpv[38, :] = 1.0
                x4 = (4.0 / TWO_PI) * viewdirs[sel].astype(np.float32)
                x4 = (x4 - np.floor(x4)) - np.float32(0.5)
                fpv[39:42, :n] = x4.T
                in_maps.append({"fpv": np.ascontiguousarray(
                                    _pack_cols(fpv, n)),
                                "wall": walls[k][0],
                                "bias": walls[k][1]})
            else:
                if zero_in is None:
                    zi = np.zeros((RB, C), np.float16)
                    zi[38, :] = 1.0
                    zero_in = {"fpv": np.ascontiguousarray(
                                   _pack_cols(zi, 0)),
                               "wall": walls[0][0],
                               "bias": walls[0][1]}
                in_maps.append(zero_in)
        global _LAST_IN_MAPS
        _LAST_IN_MAPS = in_maps
        res = None
        for attempt in range(3):
            try:
                res = run_bass_kernel_spmd(nc, in_maps,
                                           core_ids=list(range(8)))
                break
            except Exception:
                if attempt == 2:
                    raise
        assert res is not None
        for ci, (k, sel) in enumerate(batch):
            y64 = np.asarray(res.results[ci]["y"], np.float32)  # [64, COLS]
            y32 = _unpack_cols(y64)
            out[sel] = y32[:, :len(sel)].T
    return out


# revision 23
# speedup vs baseline: 1.0190x; 1.0190x over previous
"""MoE post-processing MLP kernel for Trainium2 (8 NeuronCores).

Strategy: expert-parallel sharding. Each core is assigned one chunk of
samples routed to a single expert (K=8 experts ~= 8 cores for uniform
routing). The host gathers/permutes samples by expert and the device
runs a dense 3-layer MLP entirely in fp16 (fp32 PSUM accumulation).

Posenc on device: u36 = R^T @ fpv computes all angle/2pi rows (+phase
+0.5) on the PE; v = floormod(u,1)-0.5 on Vector (single dual-op
tensor_scalar); s36 = Sin(2pi*v) on Scalar (LUT domain [-pi,pi]).
To keep fp16 phase error small at the largest scales, the host uploads
x4~ = mod(4*view/2pi,1)-0.5 (3 extra rows/block); the m=4 row uses it
with coefficient 1 and m=8 with coefficient 2 (2*x mod 1 == (2x) mod 1).

Device layout (C=8704 samples: 8 pair-packed 512-col tiles + one
256-col tail tile; the tail group runs FIRST for fast pipeline fill):
  h0 = relu(W0a^T@fpv + W0s^T@s36 + b0); h1 = relu(W1^T@h0 + b1)
  y = W2^T@h1 + b2
relu0 on Scalar, relu1 split Scalar/Vector, y-copy on Vector.
DMA rings: SP carries the small fill set (tail tile + weights) and the
outputs; the two bulk fpv halves go one each on the GpSimd software
ring and the Scalar hardware ring, so no consumer ever waits behind an
unrelated bulk transfer (ring waits are cumulative per queue).
A zero-weight matmul chain into the first PSUM group keeps the PE busy
from program start so its clock ramps to max before the real work.
"""

import numpy as np

K = 8
WID = 64
D = 32
NT = 512            # full-tile matmul moving dim (one fp32 PSUM bank)
NFULL = 8           # full tiles (1024 samples each, pair-packed)
TNT = 256           # tail-tile moving dim (512 samples)
C = NFULL * 2 * NT + 2 * TNT     # 8704 samples per core-chunk
COLS = NFULL * NT + TNT          # 4352 device columns
TWO_PI = float(2.0 * np.pi)
RB = 42             # input rows per block: feat32 pos3 view3 ones1 x4~3
CMAGIC = 12582912.0  # 1.5 * 2**23, round-to-nearest magic constant

# processing order: tail group first (tiny -> fast fill), then the four
# 1024-wide pair groups; (col0, fw) in device column space
PGROUPS = [(0, 1024), (1024, 1024), (2048, 1024), (3072, 1024),
           (4096, 256)]

# W0 row indices (DIN=74 layout: feat 0:32, posenc(pos,2) 32:47,
# posenc(view,4) 47:74) for the identity part and the sin part.
_W0A_ROWS = list(range(32)) + [32, 33, 34] + [47, 48, 49]
_W0S_ROWS = (list(range(35, 41)) + list(range(50, 62))
             + list(range(41, 47)) + list(range(62, 74)))

_PREP = None  # compiled Bass program, built once per process
_LAST_IN_MAPS = None  # stashed for external profiling harnesses


def _build_R():
    """R' [RB, 36]: u = m*x/2pi + 0.25*phase + 0.5 per angle column.
    Rows: feat(unused) 0:32, pos 32:35, view 35:38, ones 38, x4~ 39:42.
    The m=4/m=8 view columns read the host-prereduced x4~ rows."""
    Rp = np.zeros((RB, 36), np.float32)
    col = 0
    for p in range(2):
        base_phase = 0.25 * p
        for m in (1.0, 2.0):
            for c in range(3):
                Rp[32 + c, col] = m / TWO_PI
                Rp[38, col] = base_phase
                col += 1
        for m in (1.0, 2.0):
            for c in range(3):
                Rp[35 + c, col] = m / TWO_PI
                Rp[38, col] = base_phase
                col += 1
        for coef in (1.0, 2.0):
            for c in range(3):
                Rp[39 + c, col] = coef
                # x4~ stored -0.5: u = coef*(x4+0.5)+... fold into phase
                Rp[38, col] = base_phase + 0.5 * coef
                col += 1
    return Rp


def _build_program():
    import concourse.bacc as bacc
    import concourse.mybir as mybir
    from concourse.tile import TileContext

    F32, F16 = mybir.dt.float32, mybir.dt.float16
    AF = mybir.ActivationFunctionType
    ALU = mybir.AluOpType

    nc = bacc.Bacc("TRN2", target_bir_lowering=False, debug=False,
                   num_devices=8)

    fpv_d = nc.dram_tensor("fpv", [2 * RB, COLS], F16,
                           kind="ExternalInput").ap()
    wall_d = nc.dram_tensor("wall", [128, 592], F16,
                            kind="ExternalInput").ap()
    bias_d = nc.dram_tensor("bias", [128, 3], F32, kind="ExternalInput").ap()
    y_d = nc.dram_tensor("y", [64, COLS], F16, kind="ExternalOutput").ap()

    with TileContext(nc) as tc:
        with (tc.tile_pool(name="w", bufs=1) as wp,
              tc.tile_pool(name="fp", bufs=1) as fpool,
              tc.tile_pool(name="io", bufs=3) as io,
              tc.tile_pool(name="psu", bufs=2, space="PSUM") as psu,
              tc.tile_pool(name="ps0", bufs=1, space="PSUM") as ps0,
              tc.tile_pool(name="ps1", bufs=1, space="PSUM") as ps1,
              tc.tile_pool(name="psy", bufs=1, space="PSUM") as psy):
            wall = wp.tile([128, 592], F16)
            biasw = wp.tile([128, 3], F32)
            dummy = wp.tile([128, 512], F16)
            ft_A = fpool.tile([2 * RB, 2048], F16)
            ft_B = fpool.tile([2 * RB, 2048], F16)
            ft_T = fpool.tile([2 * RB, 256], F16)

            def fpv_src(c0, fw):
                if c0 >= 4096:
                    return ft_T[:, 0:fw]
                t = ft_A if c0 < 2048 else ft_B
                return t[:, c0 % 2048:c0 % 2048 + fw]

            # DMA rings: SP = fill set + outputs; GpSimd software ring =
            # first bulk half; Scalar hardware ring = second bulk half.
            nc.gpsimd.memset(dummy[:], 0.0)
            nc.gpsimd.dma_start(out=ft_A[:], in_=fpv_d[:, 0:2048])
            nc.gpsimd.dma_start(out=ft_B[:], in_=fpv_d[:, 2048:4096])
            nc.sync.dma_start(out=ft_T[:], in_=fpv_d[:, 4096:4352])
            nc.sync.dma_start(out=wall[:], in_=wall_d[:, :])
            nc.sync.dma_start(out=biasw[:], in_=bias_d[:, :])

            Rt = wall[0:2 * RB, 0:72]
            W0at = wall[0:2 * RB, 72:200]
            W0st = wall[0:72, 200:328]
            W1t = wall[0:128, 328:456]
            W2t = wall[0:128, 456:520]
            negI = wall[0:72, 520:592]
            b0t = biasw[0:128, 0:1]
            b1t = biasw[0:128, 1:2]
            b2t = biasw[0:64, 2:3]

            for gi, (gc0, fw) in enumerate(PGROUPS):
                s36 = io.tile([72, fw], F16, name="s36")
                h0t = io.tile([128, fw], F16, name="h0t")
                h1t = io.tile([128, fw], F16, name="h1t")
                yt = io.tile([64, fw], F16, name="yt")
                h0p = ps0.tile([128, fw], F32, name="h0p")
                h1p = ps1.tile([128, fw], F32, name="h1p")
                yp = psy.tile([64, fw], F32, name="yp")

                for off in range(0, fw, NT):
                    w = min(NT, fw - off)
                    js = slice(off, off + w)
                    up = psu.tile([72, NT], F32, name="up")
                    vt = io.tile([72, NT], F16, name="vt")
                    # p-state warmup / keep-warm: the HAM clock gate only
                    # reaches (and holds) 2.4 GHz under sustained PE duty,
                    # so a long zero-weight chain precedes the first real
                    # matmul and one filler pads every later tile.  All-zero
                    # weights accumulate nothing (widths vary vs dedup).
                    nd = 14 if (gi == 0 and off == 0) else 1
                    for d in range(nd):
                        nc.tensor.matmul(out=up[:, 0:NT - d],
                                         lhsT=dummy[:, 0:72],
                                         rhs=dummy[:, 0:NT - d],
                                         start=(d == 0), stop=False)
                    nc.tensor.matmul(out=up[:, 0:w], lhsT=Rt,
                                     rhs=fpv_src(gc0 + off, w),
                                     start=False, stop=False)
                    nc.vector.tensor_scalar(out=vt[:, 0:w], in0=up[:, 0:w],
                                            scalar1=CMAGIC, scalar2=CMAGIC,
                                            op0=ALU.add, op1=ALU.subtract)
                    nc.tensor.matmul(out=up[:, 0:w], lhsT=negI,
                                     rhs=vt[:, 0:w], start=False, stop=True)
                    nc.scalar.activation(s36[:, js], up[:, 0:w], AF.Sin,
                                         bias=0.0, scale=TWO_PI)
                for off in range(0, fw, NT):
                    w = min(NT, fw - off)
                    js = slice(off, off + w)
                    nc.tensor.matmul(out=h0p[:, js], lhsT=W0at,
                                     rhs=fpv_src(gc0 + off, w),
                                     start=True, stop=False)
                    nc.tensor.matmul(out=h0p[:, js], lhsT=W0st,
                                     rhs=s36[:, js], start=False, stop=True)
                nc.scalar.activation(h0t[:], h0p[:], AF.Relu,
                                     bias=b0t, scale=1.0)
                for off in range(0, fw, NT):
                    w = min(NT, fw - off)
                    js = slice(off, off + w)
                    nc.tensor.matmul(out=h1p[:, js], lhsT=W1t,
                                     rhs=h0t[:, js], start=True, stop=True)
                nc.vector.tensor_scalar(out=h1t[:], in0=h1p[:],
                                        scalar1=b1t, scalar2=0.0,
                                        op0=ALU.add, op1=ALU.max)
                for off in range(0, fw, NT):
                    w = min(NT, fw - off)
                    js = slice(off, off + w)
                    nc.tensor.matmul(out=yp[:, js], lhsT=W2t,
                                     rhs=h1t[:, js], start=True, stop=True)
                ys = (fw * 62 // 100) & ~31
                nc.scalar.activation(yt[:, 0:ys], yp[:, 0:ys],
                                     AF.Identity, bias=b2t, scale=1.0)
                nc.vector.tensor_scalar(out=yt[:, ys:fw], in0=yp[:, ys:fw],
                                        scalar1=b2t, scalar2=None,
                                        op0=ALU.add)
                nc.sync.dma_start(out=y_d[:, gc0:gc0 + fw], in_=yt[:])

    nc.compile()
    return nc


def _get_program():
    global _PREP
    if _PREP is None:
        _PREP = _build_program()
    return _PREP


def _pack_weights(W0, b0, W1, b1, W2, b2):
    """Per-expert [128, 592] fp16 weight wall + [128, 3] f32 biases."""
    W0a = np.zeros((RB, 64), np.float32)
    W0a[0:38] = W0[_W0A_ROWS]
    W0s = W0[_W0S_ROWS].astype(np.float32)          # [36, 64]
    Rp = _build_R()
    wall = np.zeros((128, 592), np.float16)
    wall[0:RB, 0:36] = Rp
    wall[RB:2 * RB, 36:72] = Rp
    wall[0:RB, 72:136] = W0a
    wall[RB:2 * RB, 136:200] = W0a
    wall[0:36, 200:264] = W0s
    wall[36:72, 264:328] = W0s
    wall[0:64, 328:392] = W1
    wall[64:128, 392:456] = W1
    wall[0:64, 456:488] = W2
    wall[64:128, 488:520] = W2
    wall[0:72, 520:592] = -np.eye(72, dtype=np.float16)
    bias = np.zeros((128, 3), np.float32)
    bias[:, 0] = np.concatenate([b0, b0])
    bias[:, 1] = np.concatenate([b1, b1])
    bias[0:64, 2] = np.concatenate([b2, b2])
    return wall, bias


def _pack_cols(data, n):
    """[R, C-samples] -> [2R, COLS] pair-packed device layout."""
    R = data.shape[0]
    full = data[:, :NFULL * 2 * NT].reshape(R, NFULL, 2, NT)
    fullp = np.concatenate([full[:, :, 0], full[:, :, 1]],
                           axis=0).reshape(2 * R, NFULL * NT)
    tail = data[:, NFULL * 2 * NT:].reshape(R, 1, 2, TNT)
    tailp = np.concatenate([tail[:, :, 0], tail[:, :, 1]],
                           axis=0).reshape(2 * R, TNT)
    return np.concatenate([fullp, tailp], axis=1)


def _unpack_cols(y):
    """[64, COLS] device layout -> [32, C] sample order."""
    yf = y[:, :NFULL * NT].reshape(64, NFULL, NT)
    full = np.stack([yf[0:32], yf[32:64]], axis=2).reshape(32, NFULL * 2 * NT)
    ytl = y[:, NFULL * NT:].reshape(64, 1, TNT)
    tail = np.stack([ytl[0:32], ytl[32:64]], axis=2).reshape(32, 2 * TNT)
    return np.concatenate([full, tail], axis=1)


def kernel(idxs, positions, viewdirs, features, W0, b0, W1, b1, W2, b2):
    from concourse.bass_utils import run_bass_kernel_spmd

    N = idxs.shape[0]
    idx = idxs.reshape(-1).astype(np.int64)
    out = np.zeros((N, D), np.float32)

    # Route: list of (expert, sample-index-array) chunks of <= C samples.
    chunks = []
    for k in range(K):
        sel = np.nonzero(idx == k)[0]
        for lo in range(0, len(sel), C):
            chunks.append((k, sel[lo:lo + C]))

    walls = [_pack_weights(W0[k], b0[k], W1[k], b1[k], W2[k], b2[k])
             for k in range(K)]

    nc = _get_program()
    zero_in = None
    for inv in range(0, len(chunks), 8):
        batch = chunks[inv:inv + 8]
        in_maps = []
        for ci in range(8):
            if ci < len(batch):
                k, sel = batch[ci]
                n = len(sel)
                fpv = np.zeros((RB, C), np.float16)
                fpv[0:32, :n] = features[sel].T
                fpv[32:35, :n] = positions[sel].T
                fpv[35:38, :n] = viewdirs[sel].T
                fpv[38, :] = 1.0
                x4 = (4.0 / TWO_PI) * viewdirs[sel].astype(np.float32)
                x4 = (x4 - np.floor(x4)) - np.float32(0.5)
                fpv[39:42, :n] = x4.T
                in_maps.append({"fpv": np.ascontiguousarray(
                                    _pack_cols(fpv, n)),
                                "wall": walls[k][0],
                                "bias": walls[k][1]})
            else:
                if zero_in is None:
                    zi = np.zeros((RB, C), np.float16)
                    zi[38, :] = 1.0
                    zero_in = {"fpv": np.ascontiguousarray(
                                   _pack_cols(zi, 0)),
                               "wall": walls[0][0],
                               "bias": walls[0][1]}
                in_maps.append(zero_in)
        global _LAST_IN_MAPS
        _LAST_IN_MAPS = in_maps
        res = None
        for attempt in range(3):
            try:
                res = run_bass_kernel_spmd(nc, in_maps,
                                           core_ids=list(range(8)))
                break
            except Exception:
                if attempt == 2:
                    raise
        assert res is not None
        for ci, (k, sel) in enumerate(batch):
            y64 = np.asarray(res.results[ci]["y"], np.float32)  # [64, COLS]
            y32 = _unpack_cols(y64)
            out[sel] = y32[:, :len(sel)].T
    return out


# revision 24
# speedup vs baseline: 1.0317x; 1.0124x over previous
"""MoE post-processing MLP kernel for Trainium2 (8 NeuronCores).

Strategy: expert-parallel sharding. Each core is assigned one chunk of
samples routed to a single expert (K=8 experts ~= 8 cores for uniform
routing). The host gathers/permutes samples by expert and the device
runs a dense 3-layer MLP entirely in fp16 (fp32 PSUM accumulation).

Posenc on device: u36 = R^T @ fpv computes all angle/2pi rows (+phase)
on the PE; Vector forms r = round(u) with the magic-constant dual-op
tensor_scalar; a -I72 matmul accumulates -r back into the same PSUM
group so Sin reads the range-reduced angles straight from PSUM
(LUT domain [-pi,pi]).
To keep fp16 phase error small at the largest scales, the host uploads
x4~ = mod(4*view/2pi,1)-0.5 (3 extra rows/block); the m=4 row uses it
with coefficient 1 and m=8 with coefficient 2 (2*x mod 1 == (2x) mod 1).

Device layout (C=8704 samples: 8 pair-packed 512-col tiles + one
256-col tail tile, processed last so the drain is short):
  h0 = relu(W0a^T@fpv + W0s^T@s36 + b0); h1 = relu(W1^T@h0 + b1)
  y = W2^T@h1 + b2
relu0 on Scalar, relu1 on Vector, y-copy split Scalar/Vector.
DMA rings: SP carries the small fill set (tail tile + weights) and the
outputs; both bulk fpv halves stream on the GpSimd software ring (it
round-robins all 16 DMA engines; SP's hardware ring is partition-
striped and ring waits are cumulative per queue, so consumers must
never sit behind an unrelated bulk transfer).
The PE's HAM clock gate only reaches 2.4 GHz after ~5.5us of sustained
duty, so a 14-matmul zero-weight chain precedes the first real matmul
and one filler matmul pads every later tile (all-zero weights
accumulated into live PSUM groups: numerically a no-op that survives
dead-code elimination).
"""

import numpy as np

K = 8
WID = 64
D = 32
NT = 512            # full-tile matmul moving dim (one fp32 PSUM bank)
NFULL = 8           # full tiles (1024 samples each, pair-packed)
TNT = 256           # tail-tile moving dim (512 samples)
C = NFULL * 2 * NT + 2 * TNT     # 8704 samples per core-chunk
COLS = NFULL * NT + TNT          # 4352 device columns
TWO_PI = float(2.0 * np.pi)
RB = 42             # input rows per block: feat32 pos3 view3 ones1 x4~3
CMAGIC = 12582912.0  # 1.5 * 2**23, round-to-nearest magic constant

# processing order: tail group first (tiny -> fast fill), then the four
# 1024-wide pair groups; (col0, fw) in device column space
PGROUPS = [(0, 1024), (1024, 1024), (2048, 1024), (3072, 1024),
           (4096, 256)]

# W0 row indices (DIN=74 layout: feat 0:32, posenc(pos,2) 32:47,
# posenc(view,4) 47:74) for the identity part and the sin part.
_W0A_ROWS = list(range(32)) + [32, 33, 34] + [47, 48, 49]
_W0S_ROWS = (list(range(35, 41)) + list(range(50, 62))
             + list(range(41, 47)) + list(range(62, 74)))

_PREP = None  # compiled Bass program, built once per process
_LAST_IN_MAPS = None  # stashed for external profiling harnesses


def _build_R():
    """R' [RB, 36]: u = m*x/2pi + 0.25*phase + 0.5 per angle column.
    Rows: feat(unused) 0:32, pos 32:35, view 35:38, ones 38, x4~ 39:42.
    The m=4/m=8 view columns read the host-prereduced x4~ rows."""
    Rp = np.zeros((RB, 36), np.float32)
    col = 0
    for p in range(2):
        base_phase = 0.25 * p
        for m in (1.0, 2.0):
            for c in range(3):
                Rp[32 + c, col] = m / TWO_PI
                Rp[38, col] = base_phase
                col += 1
        for m in (1.0, 2.0):
            for c in range(3):
                Rp[35 + c, col] = m / TWO_PI
                Rp[38, col] = base_phase
                col += 1
        for coef in (1.0, 2.0):
            for c in range(3):
                Rp[39 + c, col] = coef
                # x4~ stored -0.5: u = coef*(x4+0.5)+... fold into phase
                Rp[38, col] = base_phase + 0.5 * coef
                col += 1
    return Rp


def _build_program():
    import concourse.bacc as bacc
    import concourse.mybir as mybir
    from concourse.tile import TileContext

    F32, F16 = mybir.dt.float32, mybir.dt.float16
    AF = mybir.ActivationFunctionType
    ALU = mybir.AluOpType

    nc = bacc.Bacc("TRN2", target_bir_lowering=False, debug=False,
                   num_devices=8)

    fpv_d = nc.dram_tensor("fpv", [2 * RB, COLS], F16,
                           kind="ExternalInput").ap()
    wall_d = nc.dram_tensor("wall", [128, 592], F16,
                            kind="ExternalInput").ap()
    bias_d = nc.dram_tensor("bias", [128, 3], F32, kind="ExternalInput").ap()
    y_d = nc.dram_tensor("y", [64, COLS], F16, kind="ExternalOutput").ap()

    with TileContext(nc) as tc:
        with (tc.tile_pool(name="w", bufs=1) as wp,
              tc.tile_pool(name="fp", bufs=1) as fpool,
              tc.tile_pool(name="io", bufs=3) as io,
              tc.tile_pool(name="psu", bufs=2, space="PSUM") as psu,
              tc.tile_pool(name="ps0", bufs=1, space="PSUM") as ps0,
              tc.tile_pool(name="ps1", bufs=1, space="PSUM") as ps1,
              tc.tile_pool(name="psy", bufs=1, space="PSUM") as psy):
            wall = wp.tile([128, 592], F16)
            biasw = wp.tile([128, 3], F32)
            dummy = wp.tile([128, 512], F16)
            ft_A = fpool.tile([2 * RB, 2048], F16)
            ft_B = fpool.tile([2 * RB, 2048], F16)
            ft_T = fpool.tile([2 * RB, 256], F16)

            def fpv_src(c0, fw):
                if c0 >= 4096:
                    return ft_T[:, 0:fw]
                t = ft_A if c0 < 2048 else ft_B
                return t[:, c0 % 2048:c0 % 2048 + fw]

            # DMA rings: SP = fill set + outputs; GpSimd software ring =
            # first bulk half; Scalar hardware ring = second bulk half.
            nc.gpsimd.memset(dummy[:], 0.0)
            nc.gpsimd.dma_start(out=ft_A[:], in_=fpv_d[:, 0:2048])
            nc.gpsimd.dma_start(out=ft_B[:], in_=fpv_d[:, 2048:4096])
            nc.sync.dma_start(out=ft_T[:], in_=fpv_d[:, 4096:4352])
            nc.sync.dma_start(out=wall[:], in_=wall_d[:, :])
            nc.sync.dma_start(out=biasw[:], in_=bias_d[:, :])

            Rt = wall[0:2 * RB, 0:72]
            W0at = wall[0:2 * RB, 72:200]
            W0st = wall[0:72, 200:328]
            W1t = wall[0:128, 328:456]
            W2t = wall[0:128, 456:520]
            negI = wall[0:72, 520:592]
            b0t = biasw[0:128, 0:1]
            b1t = biasw[0:128, 1:2]
            b2t = biasw[0:64, 2:3]

            for gi, (gc0, fw) in enumerate(PGROUPS):
                s36 = io.tile([72, fw], F16, name="s36")
                h0t = io.tile([128, fw], F16, name="h0t")
                h1t = io.tile([128, fw], F16, name="h1t")
                yt = io.tile([64, fw], F16, name="yt")
                h0p = ps0.tile([128, fw], F32, name="h0p")
                h1p = ps1.tile([128, fw], F32, name="h1p")
                yp = psy.tile([64, fw], F32, name="yp")

                for off in range(0, fw, NT):
                    w = min(NT, fw - off)
                    js = slice(off, off + w)
                    up = psu.tile([72, NT], F32, name="up")
                    vt = io.tile([72, NT], F16, name="vt")
                    # p-state warmup / keep-warm: the HAM clock gate only
                    # reaches (and holds) 2.4 GHz under sustained PE duty,
                    # so a long zero-weight chain precedes the first real
                    # matmul and one filler pads every later tile.  All-zero
                    # weights accumulate nothing (widths vary vs dedup).
                    nd = 14 if (gi == 0 and off == 0) else 1
                    for d in range(nd):
                        nc.tensor.matmul(out=up[:, 0:NT - d],
                                         lhsT=dummy[:, 0:72],
                                         rhs=dummy[:, 0:NT - d],
                                         start=(d == 0), stop=False)
                    nc.tensor.matmul(out=up[:, 0:w], lhsT=Rt,
                                     rhs=fpv_src(gc0 + off, w),
                                     start=False, stop=False)
                    nc.vector.tensor_scalar(out=vt[:, 0:w], in0=up[:, 0:w],
                                            scalar1=CMAGIC, scalar2=CMAGIC,
                                            op0=ALU.add, op1=ALU.subtract)
                    nc.tensor.matmul(out=up[:, 0:w], lhsT=negI,
                                     rhs=vt[:, 0:w], start=False, stop=True)
                    nc.scalar.activation(s36[:, js], up[:, 0:w], AF.Sin,
                                         bias=0.0, scale=TWO_PI)
                for off in range(0, fw, NT):
                    w = min(NT, fw - off)
                    js = slice(off, off + w)
                    nc.tensor.matmul(out=h0p[:, js], lhsT=W0at,
                                     rhs=fpv_src(gc0 + off, w),
                                     start=True, stop=False)
                    nc.tensor.matmul(out=h0p[:, js], lhsT=W0st,
                                     rhs=s36[:, js], start=False, stop=True)
                nc.scalar.activation(h0t[:], h0p[:], AF.Relu,
                                     bias=b0t, scale=1.0)
                for off in range(0, fw, NT):
                    w = min(NT, fw - off)
                    js = slice(off, off + w)
                    nc.tensor.matmul(out=h1p[:, js], lhsT=W1t,
                                     rhs=h0t[:, js], start=True, stop=True)
                nc.vector.tensor_scalar(out=h1t[:], in0=h1p[:],
                                        scalar1=b1t, scalar2=0.0,
                                        op0=ALU.add, op1=ALU.max)
                for off in range(0, fw, NT):
                    w = min(NT, fw - off)
                    js = slice(off, off + w)
                    nc.tensor.matmul(out=yp[:, js], lhsT=W2t,
                                     rhs=h1t[:, js], start=True, stop=True)
                ys = (fw * 62 // 100) & ~31
                nc.scalar.activation(yt[:, 0:ys], yp[:, 0:ys],
                                     AF.Identity, bias=b2t, scale=1.0)
                nc.vector.tensor_scalar(out=yt[:, ys:fw], in0=yp[:, ys:fw],
                                        scalar1=b2t, scalar2=None,
                                        op0=ALU.add)
                nc.sync.dma_start(out=y_d[:, gc0:gc0 + fw], in_=yt[:])

    nc.compile()
    return nc


def _get_program():
    global _PREP
    if _PREP is None:
        _PREP = _build_program()
    return _PREP


def _pack_weights(W0, b0, W1, b1, W2, b2):
    """Per-expert [128, 592] fp16 weight wall + [128, 3] f32 biases."""
    W0a = np.zeros((RB, 64), np.float32)
    W0a[0:38] = W0[_W0A_ROWS]
    W0s = W0[_W0S_ROWS].astype(np.float32)          # [36, 64]
    Rp = _build_R()
    wall = np.zeros((128, 592), np.float16)
    wall[0:RB, 0:36] = Rp
    wall[RB:2 * RB, 36:72] = Rp
    wall[0:RB, 72:136] = W0a
    wall[RB:2 * RB, 136:200] = W0a
    wall[0:36, 200:264] = W0s
    wall[36:72, 264:328] = W0s
    wall[0:64, 328:392] = W1
    wall[64:128, 392:456] = W1
    wall[0:64, 456:488] = W2
    wall[64:128, 488:520] = W2
    wall[0:72, 520:592] = -np.eye(72, dtype=np.float16)
    bias = np.zeros((128, 3), np.float32)
    bias[:, 0] = np.concatenate([b0, b0])
    bias[:, 1] = np.concatenate([b1, b1])
    bias[0:64, 2] = np.concatenate([b2, b2])
    return wall, bias


def _pack_cols(data, n):
    """[R, C-samples] -> [2R, COLS] pair-packed device layout."""
    R = data.shape[0]
    full = data[:, :NFULL * 2 * NT].reshape(R, NFULL, 2, NT)
    fullp = np.concatenate([full[:, :, 0], full[:, :, 1]],
                           axis=0).reshape(2 * R, NFULL * NT)
    tail = data[:, NFULL * 2 * NT:].reshape(R, 1, 2, TNT)
    tailp = np.concatenate([tail[:, :, 0], tail[:, :, 1]],
                           axis=0).reshape(2 * R, TNT)
    return np.concatenate([fullp, tailp], axis=1)


def _unpack_cols(y):
    """[64, COLS] device layout -> [32, C] sample order."""
    yf = y[:, :NFULL * NT].reshape(64, NFULL, NT)
    full = np.stack([yf[0:32], yf[32:64]], axis=2).reshape(32, NFULL * 2 * NT)
    ytl = y[:, NFULL * NT:].reshape(64, 1, TNT)
    tail = np.stack([ytl[0:32], ytl[32:64]], axis=2).reshape(32, 2 * TNT)
    return np.concatenate([full, tail], axis=1)


def kernel(idxs, positions, viewdirs, features, W0, b0, W1, b1, W2, b2):
    from concourse.bass_utils import run_bass_kernel_spmd

    N = idxs.shape[0]
    idx = idxs.reshape(-1).astype(np.int64)
    out = np.zeros((N, D), np.float32)

    # Route: list of (expert, sample-index-array) chunks of <= C samples.
    chunks = []
    for k in range(K):
        sel = np.nonzero(idx == k)[0]
        for lo in range(0, len(sel), C):
            chunks.append((k, sel[lo:lo + C]))

    walls = [_pack_weights(W0[k], b0[k], W1[k], b1[k], W2[k], b2[k])
             for k in range(K)]

    nc = _get_program()
    zero_in = None
    for inv in range(0, len(chunks), 8):
        batch = chunks[inv:inv + 8]
        in_maps = []
        for ci in range(8):
            if ci < len(batch):
                k, sel = batch[ci]
                n = len(sel)
                fpv = np.zeros((RB, C), np.float16)
                fpv[0:32, :n] = features[sel].T
                fpv[32:35, :n] = positions[sel].T
                fpv[35:38, :n] = viewdirs[sel].T
                fpv[38, :] = 1.0
                x4 = (4.0 / TWO_PI) * viewdirs[sel].astype(np.float32)
                x4 = (x4 - np.floor(x4)) - np.float32(0.5)
                fpv[39:42, :n] = x4.T
                in_maps.append({"fpv": np.ascontiguousarray(
                                    _pack_cols(fpv, n)),
                                "wall": walls[k][0],
                                "bias": walls[k][1]})
            else:
                if zero_in is None:
                    zi = np.zeros((RB, C), np.float16)
                    zi[38, :] = 1.0
                    zero_in = {"fpv": np.ascontiguousarray(
                                   _pack_cols(zi, 0)),
                               "wall": walls[0][0],
                               "bias": walls[0][1]}
                in_maps.append(zero_in)
        global _LAST_IN_MAPS
        _LAST_IN_MAPS = in_maps
        res = None
        for attempt in range(3):
            try:
                res = run_bass_kernel_spmd(nc, in_maps,
                                           core_ids=list(range(8)))
                break
            except Exception:
                if attempt == 2:
                    raise
        assert res is not None
        for ci, (k, sel) in enumerate(batch):
            y64 = np.asarray(res.results[ci]["y"], np.float32)  # [64, COLS]
            y32 = _unpack_cols(y64)
            out[sel] = y32[:, :len(sel)].T
    return out
